# revision 6
# baseline (speedup 1.0000x reference)
"""Multi-head attention (B=4, S=2048, E=1024, H=16) on 8 TRN2 NeuronCores.

Sharding: core c -> (batch b = c//2, head-half hh = c%2  => 8 heads = 512 features).

v3 design (from trace analysis of the 485us v2 run):
 - v2 was ACT-bound in P2: 22 exp ACTIVATEs/group @ ~0.69us = 15.2us > PE
   13.9us, causing PE stalls, HAM cold oscillation (~127us at 1.2GHz), and
   343us of P2.
 - v3 widens everything to head-PAIRS: scores land in one [128,1024] PSUM
   tile (2 banks, both tile-position halves), exp is ONE wide op per pair:
   ACT (1024+352)/1.2 = 1.15us (573ns/tile, was 720) for 12 pairs/group,
   DVE 2-op chain over [128,1024] = 2.4us (1.2us/tile, was 1.36) for 4
   pairs/group -> ACT 13.8us, DVE 12.0us, PE 13.9us per group: balanced.
 - ctx accumulates into one [65,1024] pair tile; softmax denominators via a
   single wide reciprocal straight from PSUM row 64 (drops 2 ACT copies),
   one wide gpsimd partition_broadcast, 2 DVE muls.
 - PE warm-up: 72 junk matmuls at t=0 keep HAM's activity window busy while
   input DMAs stream, so P1 starts at 2.4GHz; a dummy ACTIVATE preloads the
   exp table set (~2.7us) off P2's critical path.
 - V projection computed TRANSPOSED (x-tile stationary) so V lands directly
   in ctx-stationary layout [keys, head, dk]; V bias folded into host-side
   bo' = bo + Wo @ bv.
"""

import os
import sys

sys.path.insert(0, "/opt/trn_rl_repo")

import numpy as np

B, S, E, H = 4, 2048, 1024, 16
DK = E // H  # 64
NCORES = 8
F = 512  # features per core (head-half)
SCALE = 1.0 / 8.0  # 1/sqrt(DK)

# ---------------------------------------------------------------- helpers

_EXP_OPS = None


def _register_exp_ops():
    """Two custom DVE ops for exp(x/8) on raw scores |x| <= ~28:
    EXPA_ANT: q = (((c3*x + c2)*x + c1)*x + 1)^4  ~= exp(x/128)
    SQ4_ANT:  out = in^16  (4 squarings)  => exp(x/8).
    """
    global _EXP_OPS
    if _EXP_OPS is not None:
        return _EXP_OPS
    import concourse.dve_ops as dve_ops
    from concourse.dve_ops import DveOp, DveOpSpec, get_dve_sub_opcode
    from concourse.dve_spec import Spec, Src0, C0, C1, C2, One, sq, lower

    existing = {op.name: op for op in dve_ops.OPS}
    if "EXPA_ANT" in existing and "SQ4_ANT" in existing:
        _EXP_OPS = (existing["EXPA_ANT"], existing["SQ4_ANT"])
        return _EXP_OPS

    def _ref_a(in0, in1, c0, c1, c2):
        x = in0.astype(np.float32)
        q = ((x * np.float32(c2) + np.float32(c1)) * x + np.float32(c0)) * x + np.float32(1.0)
        q = q * q
        return q * q

    def _ref_sq4(in0, in1, c0, c1, c2):
        x = in0.astype(np.float32)
        for _ in range(4):
            x = x * x
        return x

    opa = DveOp(
        "EXPA_ANT",
        Spec(body=sq(sq(((Src0 * C2 + C1) * Src0 + C0) * Src0 + One)), reference=_ref_a),
        subdim=False,
        uops_sha={},
    )
    opb = DveOp(
        "SQ4_ANT",
        Spec(body=sq(sq(sq(sq(Src0)))), reference=_ref_sq4),
        subdim=False,
        uops_sha={},
    )
    for op in (opa, opb):
        dve_ops.OPS.append(op)
        dve_ops._SUB_OPCODE_FOR_NAME[op.name] = (
            max(dve_ops._SUB_OPCODE_FOR_NAME.values()) + 1
        )
        dve_ops.CUSTOM_DVE_SPECS[op.name] = op.spec
        for ver in ("v3", "v4"):
            try:
                spec_c = DveOpSpec(
                    name=op.name,
                    opcode=get_dve_sub_opcode(op.name),
                    uops=lower(op.spec, ver=ver),
                    rd1_en=False,
                )
                op.uops_sha[ver] = spec_c.sha(ver)
            except Exception:
                pass
    _EXP_OPS = (opa, opb)
    return _EXP_OPS


EXPA_CONSTS = {
    "s0": 1.0 / 512.0,
    "s1": 1.0 / (2.0 * 512.0**2),
    "imm2": 1.0 / (6.0 * 512.0**3),
}

# per-group kts whose exp pair goes to the DVE (2-op wide chain); the rest
# go to ScalarE as one wide ACTIVATE.  Edges (0,1,14,15) stay on the
# lower-latency ACT path.
_DVE_KTS = frozenset({2, 5, 8, 11, 13})

_BUILT = None  # cached compiled Bass program


def _build_program():
    global _BUILT
    if _BUILT is not None:
        return _BUILT

    import concourse.bass as bass
    import concourse.mybir as mybir
    from concourse import bacc
    from concourse.tile import TileContext

    EXPA, SQ4 = _register_exp_ops()

    F32 = mybir.dt.float32
    BF16 = mybir.dt.bfloat16
    AF = mybir.ActivationFunctionType

    nc = bacc.Bacc("TRN2", target_bir_lowering=False, debug=False, num_devices=NCORES)

    xq = nc.dram_tensor("xq", [E, S], BF16, kind="ExternalInput")
    xk = nc.dram_tensor("xk", [E, S], BF16, kind="ExternalInput")
    xv = nc.dram_tensor("xv", [E, S], BF16, kind="ExternalInput")
    wq = nc.dram_tensor("wq", [E, F], BF16, kind="ExternalInput")
    wk = nc.dram_tensor("wk", [E, F], BF16, kind="ExternalInput")
    wv = nc.dram_tensor("wv", [E, F], BF16, kind="ExternalInput")
    wo = nc.dram_tensor("wo", [F, E], BF16, kind="ExternalInput")
    bq = nc.dram_tensor("bq", [F], F32, kind="ExternalInput")
    bk = nc.dram_tensor("bk", [F], F32, kind="ExternalInput")
    out_d = nc.dram_tensor("out", [E, S], F32, kind="ExternalOutput")

    with TileContext(nc) as tc:
        with (
            tc.tile_pool(name="persist", bufs=1) as persist,
            tc.tile_pool(name="xp", bufs=2) as xp,
            tc.tile_pool(name="ptp", bufs=4) as ptp,
            tc.tile_pool(name="smp", bufs=2) as smp,
            tc.tile_pool(name="ost", bufs=4) as ostp,
        ):
            QT = persist.tile([128, 4, S], BF16)
            KT = persist.tile([128, 4, S], BF16)
            Vn = persist.tile([128, 16, 8, 65], BF16)
            CX = persist.tile([128, 4, S], BF16)

            # ---------------- P0: PE warm-up + ACT table preload ----------
            junk = persist.tile([128, 512], BF16)
            nc.vector.memset(junk, 0.0)
            jexp = persist.tile([1, 8], F32)
            nc.scalar.activation(out=jexp, in_=junk[0:1, 0:8], func=AF.Exp, scale=SCALE)
            with tc.tile_pool(name="wup", bufs=1, space="PSUM") as wup:
                wp = wup.tile([128, 512], F32)
                for _ in range(72):
                    nc.tensor.matmul(
                        wp, junk[:, 0:128], junk[:, 0:512], start=True, stop=True
                    )

            # ---------------- P1: projections ----------------
            with (
                tc.tile_pool(name="wp1", bufs=1) as wp1,
                tc.tile_pool(name="mm1", bufs=3, space="PSUM") as mm1,
            ):
                # first DMAs on the queue: what the first matmul needs
                xv_r = xv.rearrange("(ec p) s -> p ec s", p=128)
                xch_next = xp.tile([128, 8, 512], BF16, tag="x", name="xch0")
                nc.sync.dma_start(out=xch_next, in_=xv_r[:, :, 0:512])
                wv_sb = wp1.tile([128, 8, F], BF16, tag="wv")
                nc.sync.dma_start(
                    out=wv_sb, in_=wv.rearrange("(ec p) f -> p ec f", p=128)
                )
                biases = persist.tile([128, 2, 4], F32)
                for ti, bt in enumerate((bq, bk)):
                    nc.sync.dma_start(
                        out=biases[:, ti, :],
                        in_=bt.rearrange("(ft p) -> p ft", p=128),
                    )
                # ones column for the rowsum trick (V stationary col 64)
                onec = persist.tile([128, 16, 8, 1], F32)
                nc.vector.memset(onec, 1.0)
                nc.vector.tensor_copy(out=Vn[:, :, :, 64:65], in_=onec)

                wq_sb = wp1.tile([128, 8, F], BF16, tag="wq")
                wk_sb = wp1.tile([128, 8, F], BF16, tag="wk")

                # V first: produced transposed ([s, f] = ctx-stationary layout)
                for sc in range(4):
                    xch = xch_next
                    if sc < 3:
                        ssl_n = slice((sc + 1) * 512, (sc + 2) * 512)
                        xch_next = xp.tile(
                            [128, 8, 512], BF16, tag="x", name=f"xch{sc+1}"
                        )
                        nc.sync.dma_start(out=xch_next, in_=xv_r[:, :, ssl_n])
                    for st in range(4):
                        stsl = slice(st * 128, (st + 1) * 128)
                        p = mm1.tile([128, 512], F32, tag="mm")
                        for ec in range(8):
                            nc.tensor.matmul(
                                p,
                                xch[:, ec, stsl],
                                wv_sb[:, ec, :],
                                start=(ec == 0),
                                stop=(ec == 7),
                            )
                        kti = sc * 4 + st
                        nc.vector.tensor_copy(
                            out=Vn[:, kti, :, 0:64],
                            in_=p.rearrange("p (h d) -> p h d", h=8),
                        )
                    if sc == 0:
                        # issue Q/K weight loads while V computes
                        nc.sync.dma_start(
                            out=wq_sb, in_=wq.rearrange("(ec p) f -> p ec f", p=128)
                        )
                        nc.sync.dma_start(
                            out=wk_sb, in_=wk.rearrange("(ec p) f -> p ec f", p=128)
                        )

                # Q, K: W stationary, x moving; bias added on eviction (ScalarE)
                for ti, (wsb, xt, dst) in enumerate(
                    ((wq_sb, xq, QT), (wk_sb, xk, KT))
                ):
                    xt_r = xt.rearrange("(ec p) s -> p ec s", p=128)
                    for sc in range(4):
                        ssl = slice(sc * 512, (sc + 1) * 512)
                        xch = xp.tile([128, 8, 512], BF16, tag="x")
                        nc.sync.dma_start(out=xch, in_=xt_r[:, :, ssl])
                        for ft in range(4):
                            fsl = slice(ft * 128, (ft + 1) * 128)
                            p = mm1.tile([128, 512], F32, tag="mm")
                            for ec in range(8):
                                nc.tensor.matmul(
                                    p,
                                    wsb[:, ec, fsl],
                                    xch[:, ec, :],
                                    start=(ec == 0),
                                    stop=(ec == 7),
                                )
                            nc.scalar.add(
                                out=dst[:, ft, ssl],
                                in_=p,
                                add=biases[:, ti, ft : ft + 1],
                            )

            # ---------------- P2: attention ----------------
            with tc.tile_pool(name="wp2", bufs=1) as wp2:
              wo_sb = wp2.tile([128, 4, E], BF16, tag="wo")
              nc.sync.dma_start(
                  out=wo_sb, in_=wo.rearrange("(fc p) e -> p fc e", p=128)
              )
              with (
                tc.tile_pool(name="scp", bufs=2, space="PSUM") as scp,
                tc.tile_pool(name="cxp", bufs=2, space="PSUM") as cxp,
              ):
                for qb in range(4):
                    qsl = slice(qb * 512, (qb + 1) * 512)
                    for pr in range(4):
                        cp = cxp.tile([65, 1024], F32, tag="cx")

                        def scores(kt):
                            ksl = slice(kt * 128, (kt + 1) * 128)
                            sp = scp.tile(
                                [128, 1024], F32, tag="sc", name=f"sp_{kt}"
                            )
                            nc.tensor.matmul(
                                sp[:, 0:512],
                                KT[0:64, pr, ksl], QT[0:64, pr, qsl],
                                start=True, stop=True, tile_position=(0, 0),
                            )
                            nc.tensor.matmul(
                                sp[:, 512:1024],
                                KT[64:128, pr, ksl], QT[64:128, pr, qsl],
                                start=True, stop=True, tile_position=(64, 0),
                            )
                            return sp

                        def exp_pair(sp, kt):
                            ptt = ptp.tile(
                                [128, 1024], BF16, tag="pt", bufs=4, name=f"pt_{kt}"
                            )
                            if kt in _DVE_KTS:
                                escr = ptp.tile(
                                    [128, 1024], F32, tag="escr", bufs=2,
                                    name=f"escr_{kt}",
                                )
                                nc.vector._custom_dve(
                                    EXPA, out=escr, in0=sp, **EXPA_CONSTS
                                )
                                nc.vector._custom_dve(SQ4, out=ptt, in0=escr)
                            else:
                                nc.scalar.activation(
                                    out=ptt, in_=sp, func=AF.Exp, scale=SCALE
                                )
                            return ptt

                        sp_ = scores(0)
                        for kt in range(16):
                            ptt = exp_pair(sp_, kt)
                            if kt < 15:
                                sp_ = scores(kt + 1)
                            nc.tensor.matmul(
                                cp[:, 0:512], Vn[:, kt, 2 * pr, :], ptt[:, 0:512],
                                start=(kt == 0), stop=(kt == 15),
                            )
                            nc.tensor.matmul(
                                cp[:, 512:1024], Vn[:, kt, 2 * pr + 1, :],
                                ptt[:, 512:1024],
                                start=(kt == 0), stop=(kt == 15),
                            )

                        # normalize: CX[:, pr, qsl] = ctx / rowsum
                        # (rowsum row must leave PSUM via ScalarE: DVE ops
                        # cannot do the partition-64 -> 0 move from PSUM)
                        sums = smp.tile([1, 1024], F32, tag="sums")
                        nc.scalar.copy(out=sums, in_=cp[64:65, 0:1024])
                        inv = smp.tile([1, 1024], F32, tag="inv")
                        nc.vector.reciprocal_approx_fast(out=inv, in_=sums)
                        invB = smp.tile([64, 1024], F32, tag="invB")
                        nc.gpsimd.partition_broadcast(out_ap=invB, in_ap=inv)
                        nc.vector.tensor_mul(
                            CX[0:64, pr, qsl], cp[0:64, 0:512], invB[:, 0:512]
                        )
                        nc.vector.tensor_mul(
                            CX[64:128, pr, qsl], cp[0:64, 512:1024],
                            invB[:, 512:1024]
                        )

              # ---------------- P3: output projection ----------------
              with tc.tile_pool(name="mmo", bufs=4, space="PSUM") as mmo:
                  for qb in range(4):
                      qsl = slice(qb * 512, (qb + 1) * 512)
                      for et in range(8):
                          esl = slice(et * 128, (et + 1) * 128)
                          p = mmo.tile([128, 512], F32, tag="mm")
                          for fc in range(4):
                              nc.tensor.matmul(
                                  p, wo_sb[:, fc, esl], CX[:, fc, qsl],
                                  start=(fc == 0), stop=(fc == 3),
                              )
                          o = ostp.tile([128, 512], F32, tag="ost")
                          if (qb * 8 + et) % 2 == 0:
                              nc.scalar.copy(out=o, in_=p)
                          else:
                              nc.vector.tensor_copy(out=o, in_=p)
                          nc.sync.dma_start(out=out_d[esl, qsl], in_=o)

    nc.compile()
    _BUILT = nc
    return nc


def _to_bf16(x: np.ndarray):
    import ml_dtypes

    return np.ascontiguousarray(x).astype(ml_dtypes.bfloat16)


def _make_in_maps(inputs):
    query = np.asarray(inputs["query"], dtype=np.float32)
    key_ = np.asarray(inputs["key_"], dtype=np.float32)
    value = np.asarray(inputs["value"], dtype=np.float32)
    Wq = np.asarray(inputs["Wq"], dtype=np.float32)
    bq = np.asarray(inputs["bq"], dtype=np.float32)
    Wk = np.asarray(inputs["Wk"], dtype=np.float32)
    bk = np.asarray(inputs["bk"], dtype=np.float32)
    Wv = np.asarray(inputs["Wv"], dtype=np.float32)
    Wo = np.asarray(inputs["Wo"], dtype=np.float32)

    WqT = _to_bf16(Wq.T)  # [E_in, E_out]
    WkT = _to_bf16(Wk.T)
    WvT = _to_bf16(Wv.T)
    WoT = _to_bf16(Wo.T)  # [F_in, E_out]

    in_maps = []
    for c in range(NCORES):
        b = c // 2
        hh = c % 2
        fsl = slice(hh * F, (hh + 1) * F)
        in_maps.append(
            {
                "xq": _to_bf16(query[b].T),
                "xk": _to_bf16(key_[b].T),
                "xv": _to_bf16(value[b].T),
                "wq": np.ascontiguousarray(WqT[:, fsl]),
                "wk": np.ascontiguousarray(WkT[:, fsl]),
                "wv": np.ascontiguousarray(WvT[:, fsl]),
                "wo": np.ascontiguousarray(WoT[fsl, :]),
                "bq": np.ascontiguousarray(bq[fsl]),
                "bk": np.ascontiguousarray(bk[fsl]),
            }
        )
    return in_maps


def kernel(**inputs) -> np.ndarray:
    from concourse.bass_utils import run_bass_kernel_spmd

    nc = _build_program()
    in_maps = _make_in_maps(inputs)

    bv = np.asarray(inputs["bv"], dtype=np.float32)
    bo = np.asarray(inputs["bo"], dtype=np.float32)
    Wo = np.asarray(inputs["Wo"], dtype=np.float32)
    bo_prime = bo + Wo @ bv  # V-bias folded through softmax + out-proj

    res = run_bass_kernel_spmd(nc, in_maps, core_ids=list(range(NCORES)))

    out = np.empty((B, S, E), dtype=np.float32)
    for b in range(B):
        partial = res.results[2 * b]["out"] + res.results[2 * b + 1]["out"]  # [E, S]
        out[b] = partial.T + bo_prime[None, :]
    return out


# revision 9
# speedup vs baseline: 1.2468x; 1.2468x over previous
"""Multi-head attention (B=4, S=2048, E=1024, H=16) on 8 TRN2 NeuronCores.

Sharding: core c -> (batch b = c//2, head-half hh = c%2  => 8 heads = 512 features).

v3 design (from trace analysis of the 485us v2 run):
 - v2 was ACT-bound in P2: 22 exp ACTIVATEs/group @ ~0.69us = 15.2us > PE
   13.9us, causing PE stalls, HAM cold oscillation (~127us at 1.2GHz), and
   343us of P2.
 - v3 widens everything to head-PAIRS: scores land in one [128,1024] PSUM
   tile (2 banks, both tile-position halves), exp is ONE wide op per pair:
   ACT (1024+352)/1.2 = 1.15us (573ns/tile, was 720) for 12 pairs/group,
   DVE 2-op chain over [128,1024] = 2.4us (1.2us/tile, was 1.36) for 4
   pairs/group -> ACT 13.8us, DVE 12.0us, PE 13.9us per group: balanced.
 - ctx accumulates into one [65,1024] pair tile; softmax denominators via a
   single wide reciprocal straight from PSUM row 64 (drops 2 ACT copies),
   one wide gpsimd partition_broadcast, 2 DVE muls.
 - PE warm-up: 72 junk matmuls at t=0 keep HAM's activity window busy while
   input DMAs stream, so P1 starts at 2.4GHz; a dummy ACTIVATE preloads the
   exp table set (~2.7us) off P2's critical path.
 - V projection computed TRANSPOSED (x-tile stationary) so V lands directly
   in ctx-stationary layout [keys, head, dk]; V bias folded into host-side
   bo' = bo + Wo @ bv.
"""

import os
import sys

sys.path.insert(0, "/opt/trn_rl_repo")

import numpy as np

B, S, E, H = 4, 2048, 1024, 16
DK = E // H  # 64
NCORES = 8
F = 512  # features per core (head-half)
SCALE = 1.0 / 8.0  # 1/sqrt(DK)

# ---------------------------------------------------------------- helpers

_EXP_OPS = None


def _register_exp_ops():
    """Two custom DVE ops for exp(x/8) on raw scores |x| <= ~28:
    EXPA_ANT: q = (((c3*x + c2)*x + c1)*x + 1)^4  ~= exp(x/128)
    SQ4_ANT:  out = in^16  (4 squarings)  => exp(x/8).
    """
    global _EXP_OPS
    if _EXP_OPS is not None:
        return _EXP_OPS
    import concourse.dve_ops as dve_ops
    from concourse.dve_ops import DveOp, DveOpSpec, get_dve_sub_opcode
    from concourse.dve_spec import Spec, Src0, C0, C1, C2, One, sq, lower

    existing = {op.name: op for op in dve_ops.OPS}
    if "EXPA_ANT" in existing and "SQ4_ANT" in existing:
        _EXP_OPS = (existing["EXPA_ANT"], existing["SQ4_ANT"])
        return _EXP_OPS

    def _ref_a(in0, in1, c0, c1, c2):
        x = in0.astype(np.float32)
        q = ((x * np.float32(c2) + np.float32(c1)) * x + np.float32(c0)) * x + np.float32(1.0)
        q = q * q
        return q * q

    def _ref_sq4(in0, in1, c0, c1, c2):
        x = in0.astype(np.float32)
        for _ in range(4):
            x = x * x
        return x

    opa = DveOp(
        "EXPA_ANT",
        Spec(body=sq(sq(((Src0 * C2 + C1) * Src0 + C0) * Src0 + One)), reference=_ref_a),
        subdim=False,
        uops_sha={},
    )
    opb = DveOp(
        "SQ4_ANT",
        Spec(body=sq(sq(sq(sq(Src0)))), reference=_ref_sq4),
        subdim=False,
        uops_sha={},
    )
    for op in (opa, opb):
        dve_ops.OPS.append(op)
        dve_ops._SUB_OPCODE_FOR_NAME[op.name] = (
            max(dve_ops._SUB_OPCODE_FOR_NAME.values()) + 1
        )
        dve_ops.CUSTOM_DVE_SPECS[op.name] = op.spec
        for ver in ("v3", "v4"):
            try:
                spec_c = DveOpSpec(
                    name=op.name,
                    opcode=get_dve_sub_opcode(op.name),
                    uops=lower(op.spec, ver=ver),
                    rd1_en=False,
                )
                op.uops_sha[ver] = spec_c.sha(ver)
            except Exception:
                pass
    _EXP_OPS = (opa, opb)
    return _EXP_OPS


EXPA_CONSTS = {
    "s0": 1.0 / 512.0,
    "s1": 1.0 / (2.0 * 512.0**2),
    "imm2": 1.0 / (6.0 * 512.0**3),
}

# per-group kts whose exp pair goes to the DVE (2-op wide chain); the rest
# go to ScalarE as one wide ACTIVATE.  Edges (0,1,14,15) stay on the
# lower-latency ACT path.
_DVE_KTS = frozenset({1, 4, 7, 10, 13})

_BUILT = None  # cached compiled Bass program


def _build_program():
    global _BUILT
    if _BUILT is not None:
        return _BUILT

    import concourse.bass as bass
    import concourse.mybir as mybir
    from concourse import bacc
    from concourse.tile import TileContext

    EXPA, SQ4 = _register_exp_ops()

    F32 = mybir.dt.float32
    BF16 = mybir.dt.bfloat16
    AF = mybir.ActivationFunctionType

    nc = bacc.Bacc("TRN2", target_bir_lowering=False, debug=False, num_devices=NCORES)

    xq = nc.dram_tensor("xq", [E, S], BF16, kind="ExternalInput")
    xk = nc.dram_tensor("xk", [E, S], BF16, kind="ExternalInput")
    xv = nc.dram_tensor("xv", [E, S], BF16, kind="ExternalInput")
    wq = nc.dram_tensor("wq", [E, F], BF16, kind="ExternalInput")
    wk = nc.dram_tensor("wk", [E, F], BF16, kind="ExternalInput")
    wv = nc.dram_tensor("wv", [E, F], BF16, kind="ExternalInput")
    wo = nc.dram_tensor("wo", [F, E], BF16, kind="ExternalInput")
    bq = nc.dram_tensor("bq", [F], F32, kind="ExternalInput")
    bk = nc.dram_tensor("bk", [F], F32, kind="ExternalInput")
    out_d = nc.dram_tensor("out", [E, S], F32, kind="ExternalOutput")

    with TileContext(nc) as tc:
        with (
            tc.tile_pool(name="persist", bufs=1) as persist,
            tc.tile_pool(name="xp", bufs=2) as xp,
            tc.tile_pool(name="ptp", bufs=4) as ptp,
            tc.tile_pool(name="smp", bufs=2) as smp,
            tc.tile_pool(name="ost", bufs=4) as ostp,
        ):
            QT = persist.tile([128, 4, S], BF16)
            KT = persist.tile([128, 4, S], BF16)
            Vn = persist.tile([128, 16, 8, 65], BF16)
            CX = persist.tile([128, 4, S], BF16)

            # ---------------- P0: ACT exp-table preload ----------
            junk = persist.tile([1, 8], BF16)
            nc.vector.memset(junk, 0.0)
            jexp = persist.tile([1, 8], F32)
            nc.scalar.activation(out=jexp, in_=junk, func=AF.Exp, scale=SCALE)

            # ---------------- P1: projections ----------------
            with (
                tc.tile_pool(name="wp1", bufs=1) as wp1,
                tc.tile_pool(name="mm1", bufs=3, space="PSUM") as mm1,
            ):
                # first DMAs on the queue: what the first matmul needs
                xv_r = xv.rearrange("(ec p) s -> p ec s", p=128)
                xch_next = xp.tile([128, 8, 512], BF16, tag="x", name="xch0")
                nc.sync.dma_start(out=xch_next, in_=xv_r[:, :, 0:512])
                wv_sb = wp1.tile([128, 8, F], BF16, tag="wv")
                nc.sync.dma_start(
                    out=wv_sb, in_=wv.rearrange("(ec p) f -> p ec f", p=128)
                )
                biases = persist.tile([128, 2, 4], F32)
                for ti, bt in enumerate((bq, bk)):
                    nc.sync.dma_start(
                        out=biases[:, ti, :],
                        in_=bt.rearrange("(ft p) -> p ft", p=128),
                    )
                # ones column for the rowsum trick (V stationary col 64)
                onec = persist.tile([128, 16, 8, 1], F32)
                nc.vector.memset(onec, 1.0)
                nc.vector.tensor_copy(out=Vn[:, :, :, 64:65], in_=onec)

                wq_sb = wp1.tile([128, 8, F], BF16, tag="wq")
                wk_sb = wp1.tile([128, 8, F], BF16, tag="wk")

                # V first: produced transposed ([s, f] = ctx-stationary layout)
                for sc in range(4):
                    xch = xch_next
                    if sc < 3:
                        ssl_n = slice((sc + 1) * 512, (sc + 2) * 512)
                        xch_next = xp.tile(
                            [128, 8, 512], BF16, tag="x", name=f"xch{sc+1}"
                        )
                        nc.sync.dma_start(out=xch_next, in_=xv_r[:, :, ssl_n])
                    for st in range(4):
                        stsl = slice(st * 128, (st + 1) * 128)
                        p = mm1.tile([128, 512], F32, tag="mm")
                        for ec in range(8):
                            nc.tensor.matmul(
                                p,
                                xch[:, ec, stsl],
                                wv_sb[:, ec, :],
                                start=(ec == 0),
                                stop=(ec == 7),
                            )
                        kti = sc * 4 + st
                        nc.vector.tensor_copy(
                            out=Vn[:, kti, :, 0:64],
                            in_=p.rearrange("p (h d) -> p h d", h=8),
                        )
                    if sc == 0:
                        # issue Q/K weight loads while V computes
                        nc.sync.dma_start(
                            out=wq_sb, in_=wq.rearrange("(ec p) f -> p ec f", p=128)
                        )
                        nc.sync.dma_start(
                            out=wk_sb, in_=wk.rearrange("(ec p) f -> p ec f", p=128)
                        )

                # Q, K: W stationary, x moving; bias added on eviction (ScalarE)
                for ti, (wsb, xt, dst) in enumerate(
                    ((wq_sb, xq, QT), (wk_sb, xk, KT))
                ):
                    xt_r = xt.rearrange("(ec p) s -> p ec s", p=128)
                    for sc in range(4):
                        ssl = slice(sc * 512, (sc + 1) * 512)
                        xch = xp.tile([128, 8, 512], BF16, tag="x")
                        nc.sync.dma_start(out=xch, in_=xt_r[:, :, ssl])
                        for ft in range(4):
                            fsl = slice(ft * 128, (ft + 1) * 128)
                            p = mm1.tile([128, 512], F32, tag="mm")
                            for ec in range(8):
                                nc.tensor.matmul(
                                    p,
                                    wsb[:, ec, fsl],
                                    xch[:, ec, :],
                                    start=(ec == 0),
                                    stop=(ec == 7),
                                )
                            nc.scalar.add(
                                out=dst[:, ft, ssl],
                                in_=p,
                                add=biases[:, ti, ft : ft + 1],
                            )

            # ---------------- P2: attention ----------------
            with tc.tile_pool(name="wp2", bufs=1) as wp2:
              wo_sb = wp2.tile([128, 4, E], BF16, tag="wo")
              nc.sync.dma_start(
                  out=wo_sb, in_=wo.rearrange("(fc p) e -> p fc e", p=128)
              )
              with (
                tc.tile_pool(name="scp", bufs=2, space="PSUM") as scp,
                tc.tile_pool(name="cxp", bufs=2, space="PSUM") as cxp,
              ):
                def scores(pr, qsl, kt):
                    ksl = slice(kt * 128, (kt + 1) * 128)
                    sp = scp.tile([128, 1024], F32, tag="sc", name=f"sp_{kt}")
                    nc.tensor.matmul(
                        sp[:, 0:512],
                        KT[0:64, pr, ksl], QT[0:64, pr, qsl],
                        start=True, stop=True, tile_position=(0, 0),
                    )
                    nc.tensor.matmul(
                        sp[:, 512:1024],
                        KT[64:128, pr, ksl], QT[64:128, pr, qsl],
                        start=True, stop=True, tile_position=(64, 0),
                    )
                    return sp

                def exp_pair(sp, kt):
                    ptt = ptp.tile(
                        [128, 1024], BF16, tag="pt", bufs=8, name=f"pt_{kt}"
                    )
                    if kt in _DVE_KTS:
                        escr = ptp.tile(
                            [128, 1024], F32, tag="escr", bufs=2,
                            name=f"escr_{kt}",
                        )
                        nc.vector._custom_dve(
                            EXPA, out=escr, in0=sp, **EXPA_CONSTS
                        )
                        nc.vector._custom_dve(SQ4, out=ptt, in0=escr)
                    else:
                        nc.scalar.activation(
                            out=ptt, in_=sp, func=AF.Exp, scale=SCALE
                        )
                    return ptt

                def emit_ctx(job):
                    cp, pr, qsl, kt, ptt = job
                    nc.tensor.matmul(
                        cp[:, 0:512], Vn[:, kt, 2 * pr, :], ptt[:, 0:512],
                        start=(kt == 0), stop=(kt == 15),
                    )
                    nc.tensor.matmul(
                        cp[:, 512:1024], Vn[:, kt, 2 * pr + 1, :],
                        ptt[:, 512:1024],
                        start=(kt == 0), stop=(kt == 15),
                    )
                    if kt == 15:
                        # normalize: CX[:, pr, qsl] = ctx / rowsum
                        # (rowsum row must leave PSUM via ScalarE: DVE ops
                        # cannot do the partition-64 -> 0 move from PSUM)
                        sums = smp.tile([1, 1024], F32, tag="sums")
                        nc.scalar.copy(out=sums, in_=cp[64:65, 0:1024])
                        inv = smp.tile([1, 1024], F32, tag="inv")
                        nc.vector.reciprocal_approx_fast(out=inv, in_=sums)
                        invB = smp.tile([64, 1024], F32, tag="invB")
                        nc.gpsimd.partition_broadcast(out_ap=invB, in_ap=inv)
                        nc.vector.tensor_mul(
                            CX[0:64, pr, qsl], cp[0:64, 0:512], invB[:, 0:512]
                        )
                        nc.vector.tensor_mul(
                            CX[64:128, pr, qsl], cp[0:64, 512:1024],
                            invB[:, 512:1024]
                        )

                # ctx trails scores/exp by LAG kts: by the time a ctx pair
                # issues, its exp (pt in SBUF, deep pool) finished long ago,
                # so the PE never waits on an in-flight ACT/DVE op.
                LAG = 5
                pending = []
                for qb in range(4):
                    qsl = slice(qb * 512, (qb + 1) * 512)
                    for pr in range(4):
                        cp = cxp.tile([65, 1024], F32, tag="cx")
                        sp_ = scores(pr, qsl, 0)
                        for kt in range(16):
                            ptt = exp_pair(sp_, kt)
                            if kt < 15:
                                sp_ = scores(pr, qsl, kt + 1)
                            pending.append((cp, pr, qsl, kt, ptt))
                            while len(pending) > LAG:
                                emit_ctx(pending.pop(0))
                while pending:
                    emit_ctx(pending.pop(0))

              # ---------------- P3: output projection ----------------
              with tc.tile_pool(name="mmo", bufs=4, space="PSUM") as mmo:
                  for qb in range(4):
                      qsl = slice(qb * 512, (qb + 1) * 512)
                      for et in range(8):
                          esl = slice(et * 128, (et + 1) * 128)
                          p = mmo.tile([128, 512], F32, tag="mm")
                          for fc in range(4):
                              nc.tensor.matmul(
                                  p, wo_sb[:, fc, esl], CX[:, fc, qsl],
                                  start=(fc == 0), stop=(fc == 3),
                              )
                          o = ostp.tile([128, 512], F32, tag="ost")
                          if (qb * 8 + et) % 2 == 0:
                              nc.scalar.copy(out=o, in_=p)
                          else:
                              nc.vector.tensor_copy(out=o, in_=p)
                          nc.sync.dma_start(out=out_d[esl, qsl], in_=o)

    nc.compile()
    _BUILT = nc
    return nc


def _to_bf16(x: np.ndarray):
    import ml_dtypes

    return np.ascontiguousarray(x).astype(ml_dtypes.bfloat16)


def _make_in_maps(inputs):
    query = np.asarray(inputs["query"], dtype=np.float32)
    key_ = np.asarray(inputs["key_"], dtype=np.float32)
    value = np.asarray(inputs["value"], dtype=np.float32)
    Wq = np.asarray(inputs["Wq"], dtype=np.float32)
    bq = np.asarray(inputs["bq"], dtype=np.float32)
    Wk = np.asarray(inputs["Wk"], dtype=np.float32)
    bk = np.asarray(inputs["bk"], dtype=np.float32)
    Wv = np.asarray(inputs["Wv"], dtype=np.float32)
    Wo = np.asarray(inputs["Wo"], dtype=np.float32)

    WqT = _to_bf16(Wq.T)  # [E_in, E_out]
    WkT = _to_bf16(Wk.T)
    WvT = _to_bf16(Wv.T)
    WoT = _to_bf16(Wo.T)  # [F_in, E_out]

    in_maps = []
    for c in range(NCORES):
        b = c // 2
        hh = c % 2
        fsl = slice(hh * F, (hh + 1) * F)
        in_maps.append(
            {
                "xq": _to_bf16(query[b].T),
                "xk": _to_bf16(key_[b].T),
                "xv": _to_bf16(value[b].T),
                "wq": np.ascontiguousarray(WqT[:, fsl]),
                "wk": np.ascontiguousarray(WkT[:, fsl]),
                "wv": np.ascontiguousarray(WvT[:, fsl]),
                "wo": np.ascontiguousarray(WoT[fsl, :]),
                "bq": np.ascontiguousarray(bq[fsl]),
                "bk": np.ascontiguousarray(bk[fsl]),
            }
        )
    return in_maps


def kernel(**inputs) -> np.ndarray:
    from concourse.bass_utils import run_bass_kernel_spmd

    nc = _build_program()
    in_maps = _make_in_maps(inputs)

    bv = np.asarray(inputs["bv"], dtype=np.float32)
    bo = np.asarray(inputs["bo"], dtype=np.float32)
    Wo = np.asarray(inputs["Wo"], dtype=np.float32)
    bo_prime = bo + Wo @ bv  # V-bias folded through softmax + out-proj

    res = run_bass_kernel_spmd(nc, in_maps, core_ids=list(range(NCORES)))

    out = np.empty((B, S, E), dtype=np.float32)
    for b in range(B):
        partial = res.results[2 * b]["out"] + res.results[2 * b + 1]["out"]  # [E, S]
        out[b] = partial.T + bo_prime[None, :]
    return out


# revision 15
# speedup vs baseline: 1.2807x; 1.0271x over previous
"""Multi-head attention (B=4, S=2048, E=1024, H=16) on 8 TRN2 NeuronCores.

Sharding: core c -> (batch b = c//2, head-half hh = c%2  => 8 heads = 512 features).

v3 design (from trace analysis of the 485us v2 run):
 - v2 was ACT-bound in P2: 22 exp ACTIVATEs/group @ ~0.69us = 15.2us > PE
   13.9us, causing PE stalls, HAM cold oscillation (~127us at 1.2GHz), and
   343us of P2.
 - v3 widens everything to head-PAIRS: scores land in one [128,1024] PSUM
   tile (2 banks, both tile-position halves), exp is ONE wide op per pair:
   ACT (1024+352)/1.2 = 1.15us (573ns/tile, was 720) for 12 pairs/group,
   DVE 2-op chain over [128,1024] = 2.4us (1.2us/tile, was 1.36) for 4
   pairs/group -> ACT 13.8us, DVE 12.0us, PE 13.9us per group: balanced.
 - ctx accumulates into one [65,1024] pair tile; softmax denominators via a
   single wide reciprocal straight from PSUM row 64 (drops 2 ACT copies),
   one wide gpsimd partition_broadcast, 2 DVE muls.
 - PE warm-up: 72 junk matmuls at t=0 keep HAM's activity window busy while
   input DMAs stream, so P1 starts at 2.4GHz; a dummy ACTIVATE preloads the
   exp table set (~2.7us) off P2's critical path.
 - V projection computed TRANSPOSED (x-tile stationary) so V lands directly
   in ctx-stationary layout [keys, head, dk]; V bias folded into host-side
   bo' = bo + Wo @ bv.
"""

import os
import sys

sys.path.insert(0, "/opt/trn_rl_repo")

import numpy as np

B, S, E, H = 4, 2048, 1024, 16
DK = E // H  # 64
NCORES = 8
F = 512  # features per core (head-half)
SCALE = 1.0 / 8.0  # 1/sqrt(DK)

# ---------------------------------------------------------------- helpers

_EXP_OPS = None


def _register_exp_ops():
    """Two custom DVE ops for exp(x/8) on raw scores |x| <= ~28:
    EXPA_ANT: q = (((c3*x + c2)*x + c1)*x + 1)^4  ~= exp(x/128)
    SQ4_ANT:  out = in^16  (4 squarings)  => exp(x/8).
    """
    global _EXP_OPS
    if _EXP_OPS is not None:
        return _EXP_OPS
    import concourse.dve_ops as dve_ops
    from concourse.dve_ops import DveOp, DveOpSpec, get_dve_sub_opcode
    from concourse.dve_spec import Spec, Src0, C0, C1, C2, One, sq, lower

    existing = {op.name: op for op in dve_ops.OPS}
    if "EXPA_ANT" in existing and "SQ4_ANT" in existing:
        _EXP_OPS = (existing["EXPA_ANT"], existing["SQ4_ANT"])
        return _EXP_OPS

    def _ref_a(in0, in1, c0, c1, c2):
        x = in0.astype(np.float32)
        q = ((x * np.float32(c2) + np.float32(c1)) * x + np.float32(c0)) * x + np.float32(1.0)
        q = q * q
        return q * q

    def _ref_sq4(in0, in1, c0, c1, c2):
        x = in0.astype(np.float32)
        for _ in range(4):
            x = x * x
        return x

    opa = DveOp(
        "EXPA_ANT",
        Spec(body=sq(sq(((Src0 * C2 + C1) * Src0 + C0) * Src0 + One)), reference=_ref_a),
        subdim=False,
        uops_sha={},
    )
    opb = DveOp(
        "SQ4_ANT",
        Spec(body=sq(sq(sq(sq(Src0)))), reference=_ref_sq4),
        subdim=False,
        uops_sha={},
    )
    for op in (opa, opb):
        dve_ops.OPS.append(op)
        dve_ops._SUB_OPCODE_FOR_NAME[op.name] = (
            max(dve_ops._SUB_OPCODE_FOR_NAME.values()) + 1
        )
        dve_ops.CUSTOM_DVE_SPECS[op.name] = op.spec
        for ver in ("v3", "v4"):
            try:
                spec_c = DveOpSpec(
                    name=op.name,
                    opcode=get_dve_sub_opcode(op.name),
                    uops=lower(op.spec, ver=ver),
                    rd1_en=False,
                )
                op.uops_sha[ver] = spec_c.sha(ver)
            except Exception:
                pass
    _EXP_OPS = (opa, opb)
    return _EXP_OPS


EXPA_CONSTS = {
    "s0": 1.0 / 512.0,
    "s1": 1.0 / (2.0 * 512.0**2),
    "imm2": 1.0 / (6.0 * 512.0**3),
}

# per-group kts whose exp pair goes to the DVE (2-op wide chain); the rest
# go to ScalarE as one wide ACTIVATE.  Every 4th group drops one DVE pair
# so ACT (~1147ns/pair + 1147 sums-copy) and DVE (~2384ns/pair + ~2556ns
# recip+muls) average out to ~14.0us/group each.
_DVE_KTS_A = frozenset({1, 4, 7, 10, 13})
_DVE_KTS_B = frozenset({1, 4, 7, 10})

_BUILT = None  # cached compiled Bass program


def _build_program():
    global _BUILT
    if _BUILT is not None:
        return _BUILT

    import concourse.bass as bass
    import concourse.mybir as mybir
    from concourse import bacc
    from concourse.tile import TileContext

    EXPA, SQ4 = _register_exp_ops()

    F32 = mybir.dt.float32
    BF16 = mybir.dt.bfloat16
    AF = mybir.ActivationFunctionType

    nc = bacc.Bacc("TRN2", target_bir_lowering=False, debug=False, num_devices=NCORES)

    xq = nc.dram_tensor("xq", [E, S], BF16, kind="ExternalInput")
    xk = nc.dram_tensor("xk", [E, S], BF16, kind="ExternalInput")
    xv = nc.dram_tensor("xv", [E, S], BF16, kind="ExternalInput")
    wq = nc.dram_tensor("wq", [E, F], BF16, kind="ExternalInput")
    wk = nc.dram_tensor("wk", [E, F], BF16, kind="ExternalInput")
    wv = nc.dram_tensor("wv", [E, F], BF16, kind="ExternalInput")
    wo = nc.dram_tensor("wo", [F, E], BF16, kind="ExternalInput")
    bq = nc.dram_tensor("bq", [F], F32, kind="ExternalInput")
    bk = nc.dram_tensor("bk", [F], F32, kind="ExternalInput")
    out_d = nc.dram_tensor("out", [E, S], BF16, kind="ExternalOutput")

    with TileContext(nc) as tc:
        with (
            tc.tile_pool(name="persist", bufs=1) as persist,
            tc.tile_pool(name="xp", bufs=2) as xp,
            tc.tile_pool(name="ptp", bufs=4) as ptp,
            tc.tile_pool(name="smp", bufs=2) as smp,
            tc.tile_pool(name="ost", bufs=4) as ostp,
        ):
            QT = persist.tile([128, 4, S], BF16)
            KT = persist.tile([128, 4, S], BF16)
            Vn = persist.tile([128, 16, 8, 65], BF16)
            CX = persist.tile([128, 4, S], BF16)

            # ---------------- P0: ACT exp-table preload ----------
            junk = persist.tile([1, 8], BF16)
            nc.vector.memset(junk, 0.0)
            jexp = persist.tile([1, 8], F32)
            nc.scalar.activation(out=jexp, in_=junk, func=AF.Exp, scale=SCALE)

            # ---------------- P1: projections ----------------
            with (
                tc.tile_pool(name="wp1", bufs=1) as wp1,
                tc.tile_pool(name="mm1", bufs=3, space="PSUM") as mm1,
            ):
                # first DMAs on the queue: what the first matmul needs
                xv_r = xv.rearrange("(ec p) s -> p ec s", p=128)
                xch_next = xp.tile([128, 8, 512], BF16, tag="x", name="xch0")
                nc.sync.dma_start(out=xch_next, in_=xv_r[:, :, 0:512])
                wv_sb = wp1.tile([128, 8, F], BF16, tag="wv")
                nc.sync.dma_start(
                    out=wv_sb, in_=wv.rearrange("(ec p) f -> p ec f", p=128)
                )
                biases = persist.tile([128, 2, 4], F32)
                for ti, bt in enumerate((bq, bk)):
                    nc.sync.dma_start(
                        out=biases[:, ti, :],
                        in_=bt.rearrange("(ft p) -> p ft", p=128),
                    )
                # ones column for the rowsum trick (V stationary col 64)
                onec = persist.tile([128, 16, 8, 1], F32)
                nc.vector.memset(onec, 1.0)
                nc.vector.tensor_copy(out=Vn[:, :, :, 64:65], in_=onec)

                wq_sb = wp1.tile([128, 8, F], BF16, tag="wq")
                wk_sb = wp1.tile([128, 8, F], BF16, tag="wk")

                # V first: produced transposed ([s, f] = ctx-stationary layout)
                for sc in range(4):
                    xch = xch_next
                    if sc < 3:
                        ssl_n = slice((sc + 1) * 512, (sc + 2) * 512)
                        xch_next = xp.tile(
                            [128, 8, 512], BF16, tag="x", name=f"xch{sc+1}"
                        )
                        nc.sync.dma_start(out=xch_next, in_=xv_r[:, :, ssl_n])
                    for st in range(4):
                        stsl = slice(st * 128, (st + 1) * 128)
                        p = mm1.tile([128, 512], F32, tag="mm")
                        for ec in range(8):
                            nc.tensor.matmul(
                                p,
                                xch[:, ec, stsl],
                                wv_sb[:, ec, :],
                                start=(ec == 0),
                                stop=(ec == 7),
                            )
                        kti = sc * 4 + st
                        nc.vector.tensor_copy(
                            out=Vn[:, kti, :, 0:64],
                            in_=p.rearrange("p (h d) -> p h d", h=8),
                        )
                    if sc == 0:
                        # issue Q/K weight loads while V computes
                        nc.sync.dma_start(
                            out=wq_sb, in_=wq.rearrange("(ec p) f -> p ec f", p=128)
                        )
                        nc.sync.dma_start(
                            out=wk_sb, in_=wk.rearrange("(ec p) f -> p ec f", p=128)
                        )

                # Q, K: W stationary, x moving; bias added on eviction (ScalarE)
                for ti, (wsb, xt, dst) in enumerate(
                    ((wq_sb, xq, QT), (wk_sb, xk, KT))
                ):
                    xt_r = xt.rearrange("(ec p) s -> p ec s", p=128)
                    for sc in range(4):
                        ssl = slice(sc * 512, (sc + 1) * 512)
                        xch = xp.tile([128, 8, 512], BF16, tag="x")
                        nc.sync.dma_start(out=xch, in_=xt_r[:, :, ssl])
                        for ft in range(4):
                            fsl = slice(ft * 128, (ft + 1) * 128)
                            p = mm1.tile([128, 512], F32, tag="mm")
                            for ec in range(8):
                                nc.tensor.matmul(
                                    p,
                                    wsb[:, ec, fsl],
                                    xch[:, ec, :],
                                    start=(ec == 0),
                                    stop=(ec == 7),
                                )
                            nc.scalar.add(
                                out=dst[:, ft, ssl],
                                in_=p,
                                add=biases[:, ti, ft : ft + 1],
                            )

            # ---------------- P2: attention ----------------
            with tc.tile_pool(name="wp2", bufs=1) as wp2:
              wo_sb = wp2.tile([128, 4, E], BF16, tag="wo")
              nc.sync.dma_start(
                  out=wo_sb, in_=wo.rearrange("(fc p) e -> p fc e", p=128)
              )
              with (
                tc.tile_pool(name="scp", bufs=2, space="PSUM") as scp,
                tc.tile_pool(name="cxp", bufs=2, space="PSUM") as cxp,
              ):
                def scores(pr, qsl, kt):
                    ksl = slice(kt * 128, (kt + 1) * 128)
                    sp = scp.tile([128, 1024], F32, tag="sc", name=f"sp_{kt}")
                    nc.tensor.matmul(
                        sp[:, 0:512],
                        KT[0:64, pr, ksl], QT[0:64, pr, qsl],
                        start=True, stop=True, tile_position=(0, 0),
                    )
                    nc.tensor.matmul(
                        sp[:, 512:1024],
                        KT[64:128, pr, ksl], QT[64:128, pr, qsl],
                        start=True, stop=True, tile_position=(64, 0),
                    )
                    return sp

                def exp_pair(sp, kt, dve_kts):
                    ptt = ptp.tile(
                        [128, 1024], BF16, tag="pt", bufs=10, name=f"pt_{kt}"
                    )
                    if kt in dve_kts:
                        escr = ptp.tile(
                            [128, 1024], F32, tag="escr", bufs=2,
                            name=f"escr_{kt}",
                        )
                        nc.vector._custom_dve(
                            EXPA, out=escr, in0=sp, **EXPA_CONSTS
                        )
                        nc.vector._custom_dve(SQ4, out=ptt, in0=escr)
                    else:
                        nc.scalar.activation(
                            out=ptt, in_=sp, func=AF.Exp, scale=SCALE
                        )
                    return ptt

                def emit_ctx(job):
                    cp, pr, qsl, kt, ptt = job
                    nc.tensor.matmul(
                        cp[:, 0:512], Vn[:, kt, 2 * pr, :], ptt[:, 0:512],
                        start=(kt == 0), stop=(kt == 15),
                    )
                    nc.tensor.matmul(
                        cp[:, 512:1024], Vn[:, kt, 2 * pr + 1, :],
                        ptt[:, 512:1024],
                        start=(kt == 0), stop=(kt == 15),
                    )
                    if kt == 15:
                        # normalize: CX[:, pr, qsl] = ctx / rowsum
                        # (rowsum row must leave PSUM via ScalarE: DVE ops
                        # cannot do the partition-64 -> 0 move from PSUM)
                        sums = smp.tile([1, 1024], F32, tag="sums")
                        nc.scalar.copy(out=sums, in_=cp[64:65, 0:1024])
                        inv = smp.tile([1, 1024], F32, tag="inv")
                        nc.vector.reciprocal_approx_fast(out=inv, in_=sums)
                        invB = smp.tile([64, 1024], F32, tag="invB")
                        nc.gpsimd.partition_broadcast(out_ap=invB, in_ap=inv)
                        nc.vector.tensor_mul(
                            CX[0:64, pr, qsl], cp[0:64, 0:512], invB[:, 0:512]
                        )
                        nc.vector.tensor_mul(
                            CX[64:128, pr, qsl], cp[0:64, 512:1024],
                            invB[:, 512:1024]
                        )

                # ctx trails scores/exp by LAG kts: by the time a ctx pair
                # issues, its exp (pt in SBUF, deep pool) finished long ago,
                # so the PE never waits on an in-flight ACT/DVE op.
                LAG = 7
                pending = []
                for qb in range(4):
                    qsl = slice(qb * 512, (qb + 1) * 512)
                    for pr in range(4):
                        g = qb * 4 + pr
                        dve_kts = _DVE_KTS_B if g % 4 == 3 else _DVE_KTS_A
                        cp = cxp.tile([65, 1024], F32, tag="cx")
                        sp_ = scores(pr, qsl, 0)
                        for kt in range(16):
                            ptt = exp_pair(sp_, kt, dve_kts)
                            if kt < 15:
                                sp_ = scores(pr, qsl, kt + 1)
                            pending.append((cp, pr, qsl, kt, ptt))
                            while len(pending) > LAG:
                                emit_ctx(pending.pop(0))
                while pending:
                    emit_ctx(pending.pop(0))

              # ---------------- P3: output projection ----------------
              with tc.tile_pool(name="mmo", bufs=4, space="PSUM") as mmo:
                  for qb in range(4):
                      qsl = slice(qb * 512, (qb + 1) * 512)
                      for et in range(8):
                          esl = slice(et * 128, (et + 1) * 128)
                          p = mmo.tile([128, 512], F32, tag="mm")
                          for fc in range(4):
                              nc.tensor.matmul(
                                  p, wo_sb[:, fc, esl], CX[:, fc, qsl],
                                  start=(fc == 0), stop=(fc == 3),
                              )
                          o = ostp.tile([128, 512], BF16, tag="ost")
                          if (qb * 8 + et) % 2 == 0:
                              nc.scalar.copy(out=o, in_=p)
                          else:
                              nc.vector.tensor_copy(out=o, in_=p)
                          nc.sync.dma_start(out=out_d[esl, qsl], in_=o)

    nc.compile()
    _BUILT = nc
    return nc


def _to_bf16(x: np.ndarray):
    import ml_dtypes

    return np.ascontiguousarray(x).astype(ml_dtypes.bfloat16)


def _make_in_maps(inputs):
    query = np.asarray(inputs["query"], dtype=np.float32)
    key_ = np.asarray(inputs["key_"], dtype=np.float32)
    value = np.asarray(inputs["value"], dtype=np.float32)
    Wq = np.asarray(inputs["Wq"], dtype=np.float32)
    bq = np.asarray(inputs["bq"], dtype=np.float32)
    Wk = np.asarray(inputs["Wk"], dtype=np.float32)
    bk = np.asarray(inputs["bk"], dtype=np.float32)
    Wv = np.asarray(inputs["Wv"], dtype=np.float32)
    Wo = np.asarray(inputs["Wo"], dtype=np.float32)

    WqT = _to_bf16(Wq.T)  # [E_in, E_out]
    WkT = _to_bf16(Wk.T)
    WvT = _to_bf16(Wv.T)
    WoT = _to_bf16(Wo.T)  # [F_in, E_out]

    in_maps = []
    for c in range(NCORES):
        b = c // 2
        hh = c % 2
        fsl = slice(hh * F, (hh + 1) * F)
        in_maps.append(
            {
                "xq": _to_bf16(query[b].T),
                "xk": _to_bf16(key_[b].T),
                "xv": _to_bf16(value[b].T),
                "wq": np.ascontiguousarray(WqT[:, fsl]),
                "wk": np.ascontiguousarray(WkT[:, fsl]),
                "wv": np.ascontiguousarray(WvT[:, fsl]),
                "wo": np.ascontiguousarray(WoT[fsl, :]),
                "bq": np.ascontiguousarray(bq[fsl]),
                "bk": np.ascontiguousarray(bk[fsl]),
            }
        )
    return in_maps


def kernel(**inputs) -> np.ndarray:
    from concourse.bass_utils import run_bass_kernel_spmd

    nc = _build_program()
    in_maps = _make_in_maps(inputs)

    bv = np.asarray(inputs["bv"], dtype=np.float32)
    bo = np.asarray(inputs["bo"], dtype=np.float32)
    Wo = np.asarray(inputs["Wo"], dtype=np.float32)
    bo_prime = bo + Wo @ bv  # V-bias folded through softmax + out-proj

    res = run_bass_kernel_spmd(nc, in_maps, core_ids=list(range(NCORES)))

    out = np.empty((B, S, E), dtype=np.float32)
    for b in range(B):
        partial = res.results[2 * b]["out"].astype(np.float32) + res.results[
            2 * b + 1
        ]["out"].astype(np.float32)  # [E, S]
        out[b] = partial.T + bo_prime[None, :]
    return out


# revision 18
# speedup vs baseline: 1.2869x; 1.0049x over previous
"""Multi-head attention (B=4, S=2048, E=1024, H=16) on 8 TRN2 NeuronCores.

Sharding: core c -> (batch b = c//2, head-half hh = c%2  => 8 heads = 512 features).

v3 design (from trace analysis of the 485us v2 run):
 - v2 was ACT-bound in P2: 22 exp ACTIVATEs/group @ ~0.69us = 15.2us > PE
   13.9us, causing PE stalls, HAM cold oscillation (~127us at 1.2GHz), and
   343us of P2.
 - v3 widens everything to head-PAIRS: scores land in one [128,1024] PSUM
   tile (2 banks, both tile-position halves), exp is ONE wide op per pair:
   ACT (1024+352)/1.2 = 1.15us (573ns/tile, was 720) for 12 pairs/group,
   DVE 2-op chain over [128,1024] = 2.4us (1.2us/tile, was 1.36) for 4
   pairs/group -> ACT 13.8us, DVE 12.0us, PE 13.9us per group: balanced.
 - ctx accumulates into one [65,1024] pair tile; softmax denominators via a
   single wide reciprocal straight from PSUM row 64 (drops 2 ACT copies),
   one wide gpsimd partition_broadcast, 2 DVE muls.
 - PE warm-up: 72 junk matmuls at t=0 keep HAM's activity window busy while
   input DMAs stream, so P1 starts at 2.4GHz; a dummy ACTIVATE preloads the
   exp table set (~2.7us) off P2's critical path.
 - V projection computed TRANSPOSED (x-tile stationary) so V lands directly
   in ctx-stationary layout [keys, head, dk]; V bias folded into host-side
   bo' = bo + Wo @ bv.
"""

import os
import sys

sys.path.insert(0, "/opt/trn_rl_repo")

import numpy as np

B, S, E, H = 4, 2048, 1024, 16
DK = E // H  # 64
NCORES = 8
F = 512  # features per core (head-half)
SCALE = 1.0 / 8.0  # 1/sqrt(DK)

# ---------------------------------------------------------------- helpers

_EXP_OPS = None


def _register_exp_ops():
    """Two custom DVE ops for exp(x/8) on raw scores |x| <= ~28:
    EXPA_ANT: q = (((c3*x + c2)*x + c1)*x + 1)^4  ~= exp(x/128)
    SQ4_ANT:  out = in^16  (4 squarings)  => exp(x/8).
    """
    global _EXP_OPS
    if _EXP_OPS is not None:
        return _EXP_OPS
    import concourse.dve_ops as dve_ops
    from concourse.dve_ops import DveOp, DveOpSpec, get_dve_sub_opcode
    from concourse.dve_spec import Spec, Src0, C0, C1, C2, One, sq, lower

    existing = {op.name: op for op in dve_ops.OPS}
    if "EXPA_ANT" in existing and "SQ4_ANT" in existing:
        _EXP_OPS = (existing["EXPA_ANT"], existing["SQ4_ANT"])
        return _EXP_OPS

    def _ref_a(in0, in1, c0, c1, c2):
        x = in0.astype(np.float32)
        q = ((x * np.float32(c2) + np.float32(c1)) * x + np.float32(c0)) * x + np.float32(1.0)
        q = q * q
        return q * q

    def _ref_sq4(in0, in1, c0, c1, c2):
        x = in0.astype(np.float32)
        for _ in range(4):
            x = x * x
        return x

    opa = DveOp(
        "EXPA_ANT",
        Spec(body=sq(sq(((Src0 * C2 + C1) * Src0 + C0) * Src0 + One)), reference=_ref_a),
        subdim=False,
        uops_sha={},
    )
    opb = DveOp(
        "SQ4_ANT",
        Spec(body=sq(sq(sq(sq(Src0)))), reference=_ref_sq4),
        subdim=False,
        uops_sha={},
    )
    for op in (opa, opb):
        dve_ops.OPS.append(op)
        dve_ops._SUB_OPCODE_FOR_NAME[op.name] = (
            max(dve_ops._SUB_OPCODE_FOR_NAME.values()) + 1
        )
        dve_ops.CUSTOM_DVE_SPECS[op.name] = op.spec
        for ver in ("v3", "v4"):
            try:
                spec_c = DveOpSpec(
                    name=op.name,
                    opcode=get_dve_sub_opcode(op.name),
                    uops=lower(op.spec, ver=ver),
                    rd1_en=False,
                )
                op.uops_sha[ver] = spec_c.sha(ver)
            except Exception:
                pass
    _EXP_OPS = (opa, opb)
    return _EXP_OPS


EXPA_CONSTS = {
    "s0": 1.0 / 512.0,
    "s1": 1.0 / (2.0 * 512.0**2),
    "imm2": 1.0 / (6.0 * 512.0**3),
}

# per-group kts whose exp pair goes to the DVE (2-op wide chain); the rest
# go to ScalarE as one wide ACTIVATE.  Every 4th group drops one DVE pair
# so ACT (~1147ns/pair + 1147 sums-copy) and DVE (~2384ns/pair + ~2556ns
# recip+muls) average out to ~14.0us/group each.
_DVE_KTS_A = frozenset({1, 4, 7, 10, 13})
_DVE_KTS_B = frozenset({1, 4, 7, 10})

_BUILT = None  # cached compiled Bass program


def _build_program():
    global _BUILT
    if _BUILT is not None:
        return _BUILT

    import concourse.bass as bass
    import concourse.mybir as mybir
    from concourse import bacc
    from concourse.tile import TileContext

    EXPA, SQ4 = _register_exp_ops()

    F32 = mybir.dt.float32
    BF16 = mybir.dt.bfloat16
    AF = mybir.ActivationFunctionType

    nc = bacc.Bacc("TRN2", target_bir_lowering=False, debug=False, num_devices=NCORES)

    xq = nc.dram_tensor("xq", [E, S], BF16, kind="ExternalInput")
    xk = nc.dram_tensor("xk", [E, S], BF16, kind="ExternalInput")
    xv = nc.dram_tensor("xv", [E, S], BF16, kind="ExternalInput")
    wq = nc.dram_tensor("wq", [E, F], BF16, kind="ExternalInput")
    wk = nc.dram_tensor("wk", [E, F], BF16, kind="ExternalInput")
    wv = nc.dram_tensor("wv", [E, F], BF16, kind="ExternalInput")
    wo = nc.dram_tensor("wo", [F, E], BF16, kind="ExternalInput")
    bq = nc.dram_tensor("bq", [F], F32, kind="ExternalInput")
    bk = nc.dram_tensor("bk", [F], F32, kind="ExternalInput")
    out_d = nc.dram_tensor("out", [E, S], BF16, kind="ExternalOutput")

    with TileContext(nc) as tc:
        with (
            tc.tile_pool(name="persist", bufs=1) as persist,
            tc.tile_pool(name="xp", bufs=2) as xp,
            tc.tile_pool(name="ptp", bufs=4) as ptp,
            tc.tile_pool(name="smp", bufs=2) as smp,
            tc.tile_pool(name="ost", bufs=4) as ostp,
        ):
            QT = persist.tile([128, 4, S], BF16)
            KT = persist.tile([128, 4, S], BF16)
            Vn = persist.tile([128, 16, 8, 65], BF16)
            CX = persist.tile([128, 4, S], BF16)

            # ---------------- P0: ACT exp-table preload ----------
            junk = persist.tile([1, 8], BF16)
            nc.vector.memset(junk, 0.0)
            jexp = persist.tile([1, 8], F32)
            nc.scalar.activation(out=jexp, in_=junk, func=AF.Exp, scale=SCALE)

            # ---------------- P1: projections ----------------
            with (
                tc.tile_pool(name="wp1", bufs=1) as wp1,
                tc.tile_pool(name="mm1", bufs=3, space="PSUM") as mm1,
            ):
                # first DMAs on the queue: what the first matmul needs
                xv_r = xv.rearrange("(ec p) s -> p ec s", p=128)
                xch_next = xp.tile([128, 8, 512], BF16, tag="x", name="xch0")
                wv_sb = wp1.tile([128, 8, F], BF16, tag="wv")
                wv_r = wv.rearrange("(ec p) f -> p ec f", p=128)
                # split the first loads so the first matmul (needs ec=0 only)
                # starts as early as possible
                nc.sync.dma_start(out=xch_next[:, 0:2, :], in_=xv_r[:, 0:2, 0:512])
                nc.sync.dma_start(out=wv_sb[:, 0:2, :], in_=wv_r[:, 0:2, :])
                nc.sync.dma_start(out=xch_next[:, 2:8, :], in_=xv_r[:, 2:8, 0:512])
                nc.sync.dma_start(out=wv_sb[:, 2:8, :], in_=wv_r[:, 2:8, :])
                biases = persist.tile([128, 2, 4], F32)
                for ti, bt in enumerate((bq, bk)):
                    nc.sync.dma_start(
                        out=biases[:, ti, :],
                        in_=bt.rearrange("(ft p) -> p ft", p=128),
                    )
                # ones column for the rowsum trick (V stationary col 64)
                onec = persist.tile([128, 16, 8, 1], F32)
                nc.vector.memset(onec, 1.0)
                nc.vector.tensor_copy(out=Vn[:, :, :, 64:65], in_=onec)

                wq_sb = wp1.tile([128, 8, F], BF16, tag="wq")
                wk_sb = wp1.tile([128, 8, F], BF16, tag="wk")

                # V first: produced transposed ([s, f] = ctx-stationary layout)
                for sc in range(4):
                    xch = xch_next
                    if sc < 3:
                        ssl_n = slice((sc + 1) * 512, (sc + 2) * 512)
                        xch_next = xp.tile(
                            [128, 8, 512], BF16, tag="x", name=f"xch{sc+1}"
                        )
                        nc.sync.dma_start(out=xch_next, in_=xv_r[:, :, ssl_n])
                    for st in range(4):
                        stsl = slice(st * 128, (st + 1) * 128)
                        p = mm1.tile([128, 512], F32, tag="mm")
                        for ec in range(8):
                            nc.tensor.matmul(
                                p,
                                xch[:, ec, stsl],
                                wv_sb[:, ec, :],
                                start=(ec == 0),
                                stop=(ec == 7),
                            )
                        kti = sc * 4 + st
                        nc.vector.tensor_copy(
                            out=Vn[:, kti, :, 0:64],
                            in_=p.rearrange("p (h d) -> p h d", h=8),
                        )
                    if sc == 0:
                        # issue Q/K weight loads while V computes
                        nc.sync.dma_start(
                            out=wq_sb, in_=wq.rearrange("(ec p) f -> p ec f", p=128)
                        )
                        nc.sync.dma_start(
                            out=wk_sb, in_=wk.rearrange("(ec p) f -> p ec f", p=128)
                        )

                # Q, K: W stationary, x moving; bias added on eviction (ScalarE)
                for ti, (wsb, xt, dst) in enumerate(
                    ((wq_sb, xq, QT), (wk_sb, xk, KT))
                ):
                    xt_r = xt.rearrange("(ec p) s -> p ec s", p=128)
                    for sc in range(4):
                        ssl = slice(sc * 512, (sc + 1) * 512)
                        xch = xp.tile([128, 8, 512], BF16, tag="x")
                        nc.sync.dma_start(out=xch, in_=xt_r[:, :, ssl])
                        for ft in range(4):
                            fsl = slice(ft * 128, (ft + 1) * 128)
                            p = mm1.tile([128, 512], F32, tag="mm")
                            for ec in range(8):
                                nc.tensor.matmul(
                                    p,
                                    wsb[:, ec, fsl],
                                    xch[:, ec, :],
                                    start=(ec == 0),
                                    stop=(ec == 7),
                                )
                            nc.scalar.add(
                                out=dst[:, ft, ssl],
                                in_=p,
                                add=biases[:, ti, ft : ft + 1],
                            )

            # ---------------- P2: attention ----------------
            with tc.tile_pool(name="wp2", bufs=1) as wp2:
              wo_sb = wp2.tile([128, 4, E], BF16, tag="wo")
              nc.sync.dma_start(
                  out=wo_sb, in_=wo.rearrange("(fc p) e -> p fc e", p=128)
              )
              with (
                tc.tile_pool(name="scp", bufs=2, space="PSUM") as scp,
                tc.tile_pool(name="cxp", bufs=2, space="PSUM") as cxp,
              ):
                def scores(pr, qsl, kt):
                    ksl = slice(kt * 128, (kt + 1) * 128)
                    sp = scp.tile([128, 1024], F32, tag="sc", name=f"sp_{kt}")
                    nc.tensor.matmul(
                        sp[:, 0:512],
                        KT[0:64, pr, ksl], QT[0:64, pr, qsl],
                        start=True, stop=True, tile_position=(0, 0),
                    )
                    nc.tensor.matmul(
                        sp[:, 512:1024],
                        KT[64:128, pr, ksl], QT[64:128, pr, qsl],
                        start=True, stop=True, tile_position=(64, 0),
                    )
                    return sp

                def exp_pair(sp, kt, dve_kts):
                    ptt = ptp.tile(
                        [128, 1024], BF16, tag="pt", bufs=13, name=f"pt_{kt}"
                    )
                    if kt in dve_kts:
                        escr = ptp.tile(
                            [128, 1024], F32, tag="escr", bufs=2,
                            name=f"escr_{kt}",
                        )
                        nc.vector._custom_dve(
                            EXPA, out=escr, in0=sp, **EXPA_CONSTS
                        )
                        nc.vector._custom_dve(SQ4, out=ptt, in0=escr)
                    else:
                        nc.scalar.activation(
                            out=ptt, in_=sp, func=AF.Exp, scale=SCALE
                        )
                    return ptt

                def emit_ctx(job):
                    cp, pr, qsl, kt, ptt = job
                    nc.tensor.matmul(
                        cp[:, 0:512], Vn[:, kt, 2 * pr, :], ptt[:, 0:512],
                        start=(kt == 0), stop=(kt == 15),
                    )
                    nc.tensor.matmul(
                        cp[:, 512:1024], Vn[:, kt, 2 * pr + 1, :],
                        ptt[:, 512:1024],
                        start=(kt == 0), stop=(kt == 15),
                    )
                    if kt == 15:
                        # normalize: CX[:, pr, qsl] = ctx / rowsum
                        # (rowsum row must leave PSUM via ScalarE: DVE ops
                        # cannot do the partition-64 -> 0 move from PSUM)
                        sums = smp.tile([1, 1024], F32, tag="sums")
                        nc.scalar.copy(out=sums, in_=cp[64:65, 0:1024])
                        inv = smp.tile([1, 1024], F32, tag="inv")
                        nc.vector.reciprocal_approx_fast(out=inv, in_=sums)
                        invB = smp.tile([64, 1024], F32, tag="invB")
                        nc.gpsimd.partition_broadcast(out_ap=invB, in_ap=inv)
                        nc.vector.tensor_mul(
                            CX[0:64, pr, qsl], cp[0:64, 0:512], invB[:, 0:512]
                        )
                        nc.vector.tensor_mul(
                            CX[64:128, pr, qsl], cp[0:64, 512:1024],
                            invB[:, 512:1024]
                        )

                # ctx trails scores/exp by LAG kts: by the time a ctx pair
                # issues, its exp (pt in SBUF, deep pool) finished long ago,
                # so the PE never waits on an in-flight ACT/DVE op.
                LAG = 10
                pending = []
                for qb in range(4):
                    qsl = slice(qb * 512, (qb + 1) * 512)
                    for pr in range(4):
                        g = qb * 4 + pr
                        dve_kts = _DVE_KTS_B if g % 4 == 3 else _DVE_KTS_A
                        cp = cxp.tile([65, 1024], F32, tag="cx")
                        sp_ = scores(pr, qsl, 0)
                        for kt in range(16):
                            ptt = exp_pair(sp_, kt, dve_kts)
                            if kt < 15:
                                sp_ = scores(pr, qsl, kt + 1)
                            pending.append((cp, pr, qsl, kt, ptt))
                            while len(pending) > LAG:
                                emit_ctx(pending.pop(0))
                while pending:
                    emit_ctx(pending.pop(0))

              # ---------------- P3: output projection ----------------
              with tc.tile_pool(name="mmo", bufs=4, space="PSUM") as mmo:
                  for qb in range(4):
                      qsl = slice(qb * 512, (qb + 1) * 512)
                      for et in range(8):
                          esl = slice(et * 128, (et + 1) * 128)
                          p = mmo.tile([128, 512], F32, tag="mm")
                          for fc in range(4):
                              nc.tensor.matmul(
                                  p, wo_sb[:, fc, esl], CX[:, fc, qsl],
                                  start=(fc == 0), stop=(fc == 3),
                              )
                          o = ostp.tile([128, 512], BF16, tag="ost")
                          if (qb * 8 + et) % 2 == 0:
                              nc.scalar.copy(out=o, in_=p)
                          else:
                              nc.vector.tensor_copy(out=o, in_=p)
                          nc.sync.dma_start(out=out_d[esl, qsl], in_=o)

    nc.compile()
    _BUILT = nc
    return nc


def _to_bf16(x: np.ndarray):
    import ml_dtypes

    return np.ascontiguousarray(x).astype(ml_dtypes.bfloat16)


def _make_in_maps(inputs):
    query = np.asarray(inputs["query"], dtype=np.float32)
    key_ = np.asarray(inputs["key_"], dtype=np.float32)
    value = np.asarray(inputs["value"], dtype=np.float32)
    Wq = np.asarray(inputs["Wq"], dtype=np.float32)
    bq = np.asarray(inputs["bq"], dtype=np.float32)
    Wk = np.asarray(inputs["Wk"], dtype=np.float32)
    bk = np.asarray(inputs["bk"], dtype=np.float32)
    Wv = np.asarray(inputs["Wv"], dtype=np.float32)
    Wo = np.asarray(inputs["Wo"], dtype=np.float32)

    WqT = _to_bf16(Wq.T)  # [E_in, E_out]
    WkT = _to_bf16(Wk.T)
    WvT = _to_bf16(Wv.T)
    WoT = _to_bf16(Wo.T)  # [F_in, E_out]

    in_maps = []
    for c in range(NCORES):
        b = c // 2
        hh = c % 2
        fsl = slice(hh * F, (hh + 1) * F)
        in_maps.append(
            {
                "xq": _to_bf16(query[b].T),
                "xk": _to_bf16(key_[b].T),
                "xv": _to_bf16(value[b].T),
                "wq": np.ascontiguousarray(WqT[:, fsl]),
                "wk": np.ascontiguousarray(WkT[:, fsl]),
                "wv": np.ascontiguousarray(WvT[:, fsl]),
                "wo": np.ascontiguousarray(WoT[fsl, :]),
                "bq": np.ascontiguousarray(bq[fsl]),
                "bk": np.ascontiguousarray(bk[fsl]),
            }
        )
    return in_maps


def kernel(**inputs) -> np.ndarray:
    from concourse.bass_utils import run_bass_kernel_spmd

    nc = _build_program()
    in_maps = _make_in_maps(inputs)

    bv = np.asarray(inputs["bv"], dtype=np.float32)
    bo = np.asarray(inputs["bo"], dtype=np.float32)
    Wo = np.asarray(inputs["Wo"], dtype=np.float32)
    bo_prime = bo + Wo @ bv  # V-bias folded through softmax + out-proj

    res = run_bass_kernel_spmd(nc, in_maps, core_ids=list(range(NCORES)))

    out = np.empty((B, S, E), dtype=np.float32)
    for b in range(B):
        partial = res.results[2 * b]["out"].astype(np.float32) + res.results[
            2 * b + 1
        ]["out"].astype(np.float32)  # [E, S]
        out[b] = partial.T + bo_prime[None, :]
    return out
